# revision 15
# baseline (speedup 1.0000x reference)
"""SAM-style windowed-attention transformer block on 8 TRN2 cores.

Data-parallel: core b processes batch element b end-to-end (no collectives).
Shapes hardcoded: x [8,64,64,768], prompt row prepended -> grid 65x64 tokens,
windows 14x14 (5x5=25 windows, padded grid 70x70), NH=12, HD=64, MLP=3072.
"""
import numpy as np
from contextlib import ExitStack

import concourse.bass as bass
import concourse.tile as tile
import concourse.mybir as mybir
from concourse import bacc
from concourse.bass_utils import run_bass_kernel_spmd
from concourse.masks import make_identity

F32 = mybir.dt.float32
F32R = mybir.dt.float32r
AF = mybir.ActivationFunctionType
OP = mybir.AluOpType
AX = mybir.AxisListType

WS = 14
NH = 12
HD = 64
C = 768
MLP = 3072
GR, GC = 65, 64          # token grid (prompt row + 64 image rows) x 64 cols
T = GR * GC              # 4160 tokens
NWIN = 5                 # windows per axis (70/14)
N = WS * WS              # 196 tokens per window
CC = C // 128            # 6 contraction chunks
QK_OD = 2 * C            # q then k, feature-major
REL = NH * WS            # 168 rel output dims (head, k-index)
QKT_STRIDE = 4224        # 4160 padded to avoid OOB on wrapped edge reads

_CACHE = {}


def _ap(base, off, dims):
    return bass.AP(tensor=base.tensor, offset=base.offset + off, ap=dims)


def build():
    nc = bacc.Bacc("TRN2", target_bir_lowering=False, debug=False)

    # ---------------- I/O ----------------
    x_b = nc.dram_tensor("x_b", [4096, C], F32, kind="ExternalInput").ap()
    prompt_b = nc.dram_tensor("prompt_b", [64, C], F32, kind="ExternalInput").ap()
    ln1_g = nc.dram_tensor("ln1_g", [1, C], F32, kind="ExternalInput").ap()
    ln1_b = nc.dram_tensor("ln1_b", [1, C], F32, kind="ExternalInput").ap()
    qkv_w = nc.dram_tensor("qkv_w", [3 * C, C], F32, kind="ExternalInput").ap()
    qkv_b = nc.dram_tensor("qkv_b", [3 * C, 1], F32, kind="ExternalInput").ap()
    rel_h_in = nc.dram_tensor("rel_h_in", [27, HD], F32, kind="ExternalInput").ap()
    rel_w_in = nc.dram_tensor("rel_w_in", [27, HD], F32, kind="ExternalInput").ap()
    proj_w = nc.dram_tensor("proj_w", [C, C], F32, kind="ExternalInput").ap()
    proj_b = nc.dram_tensor("proj_b", [1, C], F32, kind="ExternalInput").ap()
    ln2_g = nc.dram_tensor("ln2_g", [1, C], F32, kind="ExternalInput").ap()
    ln2_b = nc.dram_tensor("ln2_b", [1, C], F32, kind="ExternalInput").ap()
    mlp_w1 = nc.dram_tensor("mlp_w1", [MLP, C], F32, kind="ExternalInput").ap()
    mlp_b1 = nc.dram_tensor("mlp_b1", [MLP, 1], F32, kind="ExternalInput").ap()
    mlp_w2 = nc.dram_tensor("mlp_w2", [C, MLP], F32, kind="ExternalInput").ap()
    mlp_b2 = nc.dram_tensor("mlp_b2", [1, C], F32, kind="ExternalInput").ap()
    out = nc.dram_tensor("out", [T, C], F32, kind="ExternalOutput").ap()

    # ---------------- internal DRAM ----------------
    wqkT_d = nc.dram_tensor("wqkT_d", [C, QK_OD], F32R).ap()
    w1T_d = nc.dram_tensor("w1T_d", [C, MLP], F32R).ap()
    w2T_d = nc.dram_tensor("w2T_d", [MLP, C], F32R).ap()
    qkT_d = nc.dram_tensor("qkT_d", [QK_OD, QKT_STRIDE], F32R).ap()
    v_tm_d = nc.dram_tensor("v_tm_d", [T, C], F32R).ap()
    relh_d = nc.dram_tensor("relh_d", [T, REL], F32).ap()
    relw_d = nc.dram_tensor("relw_d", [T, REL], F32).ap()
    pb_d = nc.dram_tensor("pb_d", [54, NH], F32).ap()  # rel bias table (t, head)
    x2p_d = nc.dram_tensor("x2p_d", [70, 70, C], F32).ap()
    h1T_d = nc.dram_tensor("h1T_d", [MLP, T], F32R).ap()
    bvr_d = nc.dram_tensor("bvr_d", [1, C], F32R).ap()

    NBLK = 9  # token blocks for 512-wide moving ops (8x512 + 64)

    def blk_sl(b):
        s = b * 512
        return s, min(512, T - s)

    with tile.TileContext(nc) as tc, ExitStack() as ctx0:
        persist = ctx0.enter_context(tc.tile_pool(name="persist", bufs=1))

        # identity matrices for PE transpose
        ident = persist.tile([128, 128], F32)
        make_identity(nc, ident[:])
        ident_r = persist.tile([128, 128], F32R)
        nc.vector.tensor_copy(ident_r[:], ident[:])

        # broadcast/bias tiles
        def bcast_row(src_row, w, tag, dt=F32, pool=None):
            t_ = (pool or persist).tile([128, w], dt, tag=tag)
            nc.sync.dma_start(out=t_[:], in_=_ap(src_row, 0, [[0, 128], [1, w]]))
            return t_

        g1_t = bcast_row(ln1_g, C, "g1_t")
        b1_t = bcast_row(ln1_b, C, "b1_t")
        g2_t = bcast_row(ln2_g, C, "g2_t")
        b2_t = bcast_row(ln2_b, C, "b2_t")
        projb_t = bcast_row(proj_b, C, "projb_t")
        mlpb2_t = bcast_row(mlp_b2, C, "mlpb2_t")
        bv_t = bcast_row(_ap(qkv_b, 2 * C, [[1, 1], [1, C]]), C, "bv_t")
        bv_r = persist.tile([128, C], F32R)
        nc.vector.tensor_copy(bv_r[:], bv_t[:])
        nc.sync.dma_start(out=bvr_d, in_=bv_r[0:1, :])
        qkb_col = persist.tile([128, 12, 1], F32)   # q,k bias cols per oc
        nc.sync.dma_start(out=qkb_col[:], in_=_ap(qkv_b, 0, [[1, 128], [128, 12], [0, 1]]))
        kb_col = persist.tile([128, 6, 1], F32)     # k bias cols for slab prefill
        nc.sync.dma_start(out=kb_col[:], in_=_ap(qkv_b, C, [[1, 128], [128, 6], [0, 1]]))
        b1_col = persist.tile([128, 24, 1], F32)
        nc.sync.dma_start(out=b1_col[:], in_=_ap(mlp_b1, 0, [[1, 128], [128, 24], [0, 1]]))
        eps_t = persist.tile([128, 1], F32)
        nc.vector.memset(eps_t[:], 1e-5)

        # rel_cat^T [64, 54] (transpose gather, one-time small)
        relT = persist.tile([64, 54], F32)
        nc.sync.dma_start(out=relT[:, 0:27], in_=_ap(rel_h_in, 0, [[1, 64], [64, 27]]))
        nc.sync.dma_start(out=relT[:, 27:54], in_=_ap(rel_w_in, 0, [[1, 64], [64, 27]]))
        # bq matrix [64, 12]
        bq_m = persist.tile([64, 12], F32)
        nc.sync.dma_start(out=bq_m[:], in_=_ap(qkv_b, 0, [[1, 64], [64, 12], [0, 1]]))

        projWT = persist.tile([128, CC, C], F32R)   # proj_w^T resident
        GrT = persist.tile([128, CC, NH, 54], F32R)  # [c, cc, head, revh27|revw27]

        # =========== P0: weight transposes + rel weight build ===========
        with tc.tile_pool(name="p0", bufs=3) as p0, \
             tc.tile_pool(name="p0ps", bufs=4, space="PSUM") as p0ps:

            def wtile_T(src, oc, n_oc, dst_dram, dst_sb=None):
                # natural W rows [128 od, C] -> 6 transposes -> [128 c, 128 od]
                wn = p0.tile([128, C], F32, tag="wn")
                nc.sync.dma_start(out=wn[:], in_=src[oc * 128:(oc + 1) * 128, :])
                for cc in range(CC):
                    pt = p0ps.tile([128, 128], F32, tag="pt")
                    nc.tensor.transpose(pt[:], wn[:, cc * 128:(cc + 1) * 128], ident[:])
                    if dst_sb is not None:
                        nc.scalar.copy(dst_sb[:, cc, oc * 128:(oc + 1) * 128], pt[:])
                    else:
                        ws = p0.tile([128, 128], F32R, tag="ws")
                        nc.scalar.copy(ws[:], pt[:])
                        nc.sync.dma_start(
                            out=dst_dram[cc * 128:(cc + 1) * 128,
                                         oc * 128:(oc + 1) * 128], in_=ws[:])

            for oc in range(12):            # Q,K -> DRAM
                wtile_T(qkv_w, oc, 12, wqkT_d)
            wvT = persist.tile([128, CC, C], F32R)
            for oc in range(6):             # V -> resident (od offset 1536)
                wn = p0.tile([128, C], F32, tag="wn")
                nc.sync.dma_start(out=wn[:], in_=qkv_w[2 * C + oc * 128: 2 * C + (oc + 1) * 128, :])
                for cc in range(CC):
                    pt = p0ps.tile([128, 128], F32, tag="pt")
                    nc.tensor.transpose(pt[:], wn[:, cc * 128:(cc + 1) * 128], ident[:])
                    nc.scalar.copy(wvT[:, cc, oc * 128:(oc + 1) * 128], pt[:])
            for oc in range(6):             # proj -> resident
                wn = p0.tile([128, C], F32, tag="wn")
                nc.sync.dma_start(out=wn[:], in_=proj_w[oc * 128:(oc + 1) * 128, :])
                for cc in range(CC):
                    pt = p0ps.tile([128, 128], F32, tag="pt")
                    nc.tensor.transpose(pt[:], wn[:, cc * 128:(cc + 1) * 128], ident[:])
                    nc.scalar.copy(projWT[:, cc, oc * 128:(oc + 1) * 128], pt[:])
            for oc in range(24):            # mlp_w1 -> DRAM
                wtile_T(mlp_w1, oc, 24, w1T_d)
            for oc2 in range(6):            # mlp_w2 [768, 3072] -> w2T [3072, 768]
                wn = p0.tile([128, MLP], F32, tag="wn2")
                nc.sync.dma_start(out=wn[:], in_=mlp_w2[oc2 * 128:(oc2 + 1) * 128, :])
                for cc2 in range(24):
                    pt = p0ps.tile([128, 128], F32, tag="pt")
                    nc.tensor.transpose(pt[:], wn[:, cc2 * 128:(cc2 + 1) * 128], ident[:])
                    ws = p0.tile([128, 128], F32R, tag="ws")
                    nc.scalar.copy(ws[:], pt[:])
                    nc.sync.dma_start(
                        out=w2T_d[cc2 * 128:(cc2 + 1) * 128,
                                  oc2 * 128:(oc2 + 1) * 128], in_=ws[:])

            # G matrices: per head psum[c_chunk, 54] = Wq_head-slice^T @ relT
            for h in range(NH):
                wqn = p0.tile([64, C], F32, tag="wqn")
                nc.sync.dma_start(out=wqn[:], in_=qkv_w[h * 64:(h + 1) * 64, :])
                for cc in range(CC):
                    gp = p0ps.tile([128, 54], F32, tag="gp")
                    nc.tensor.matmul(gp[:], wqn[:, cc * 128:(cc + 1) * 128],
                                     relT[:], start=True, stop=True)
                    # store reversed: dst t' = 26-t per half
                    nc.vector.tensor_copy(
                        GrT[:, cc, h, 0:27],
                        _ap(gp[:], 26, [[54, 128], [-1, 27]]))
                    nc.vector.tensor_copy(
                        GrT[:, cc, h, 27:54],
                        _ap(gp[:], 53, [[54, 128], [-1, 27]]))
            # rel bias table: psum [54 t, 12 h] = relT^T? -> lhsT=relT[64,54], rhs=bq_m
            pbp = p0ps.tile([54, 12], F32, tag="gp")
            nc.tensor.matmul(pbp[:], relT[:], bq_m[:], start=True, stop=True)
            pbs = p0.tile([54, 12], F32, tag="pbs")
            nc.scalar.copy(pbs[:], pbp[:])
            nc.sync.dma_start(out=pb_d, in_=pbs[:])

        # =========== P1..P2: LN1, big GEMMs (x_lnT scope) ===========
        with tc.tile_pool(name="xlnT_p", bufs=1) as xlnT_p:
            x_lnT = xlnT_p.tile([128, CC, T], F32R)

            def ln_pass(src_tiles, g_t, b_t, dstT, pool, psp):
                """src_tiles: list of (n_tok, dma_fn) per 128-token tile."""
                for ti, (ntok, dma_fn) in enumerate(src_tiles):
                    xt = pool.tile([128, C], F32, tag="xt")
                    dma_fn(xt)
                    st = pool.tile([128, 3, 6], F32, tag="st")
                    for sg in range(3):
                        nc.vector.bn_stats(out=st[:ntok, sg, :],
                                           in_=xt[:ntok, sg * 256:(sg + 1) * 256])
                    mv = pool.tile([128, 2], F32, tag="mv")
                    nc.vector.bn_aggr(out=mv[:ntok], in_=st[:ntok])
                    rstd = pool.tile([128, 1], F32, tag="rstd")
                    nc.scalar.activation(out=rstd[:ntok], in_=mv[:ntok, 1:2],
                                         func=AF.Sqrt, bias=eps_t[:ntok], scale=1.0)
                    nc.vector.reciprocal(out=rstd[:ntok], in_=rstd[:ntok])
                    bmn = pool.tile([128, 1], F32, tag="bmn")
                    nc.vector.scalar_tensor_tensor(
                        out=bmn[:ntok], in0=mv[:ntok, 0:1], scalar=-1.0,
                        in1=rstd[:ntok], op0=OP.mult, op1=OP.mult)
                    xn = pool.tile([128, C], F32, tag="xn")
                    nc.scalar.activation(out=xn[:ntok], in_=xt[:ntok],
                                         func=AF.Identity, bias=bmn[:ntok],
                                         scale=rstd[:ntok])
                    nc.vector.scalar_tensor_tensor(
                        out=xn[:ntok], in0=xn[:ntok], scalar=1.0,
                        in1=g_t[:ntok], op0=OP.mult, op1=OP.mult)
                    nc.vector.tensor_add(xn[:ntok], xn[:ntok], b_t[:ntok])
                    for cc in range(CC):
                        pt = psp.tile([128, 128], F32, tag="lnpt")
                        nc.tensor.transpose(pt[:, :ntok],
                                            xn[:ntok, cc * 128:(cc + 1) * 128],
                                            ident[:ntok, :ntok])
                        nc.scalar.copy(dstT[:, cc, ti * 128: ti * 128 + ntok],
                                       pt[:, :ntok])

            with tc.tile_pool(name="ln1", bufs=3) as lnp, \
                 tc.tile_pool(name="ln1ps", bufs=4, space="PSUM") as lnps:
                tiles = []

                def mk_dma0(xt_tile):
                    nc.sync.dma_start(out=xt_tile[0:64, :], in_=prompt_b)
                    nc.sync.dma_start(out=xt_tile[64:128, :], in_=x_b[0:64, :])

                tiles.append((128, mk_dma0))
                for ti in range(1, 33):
                    ntok = 128 if ti < 32 else 64
                    def mk(ti=ti, ntok=ntok):
                        def f(xt_tile):
                            nc.sync.dma_start(
                                out=xt_tile[:ntok, :],
                                in_=x_b[ti * 128 - 64: ti * 128 - 64 + ntok, :])
                        return f
                    tiles.append((ntok, mk()))
                ln_pass(tiles, g1_t, b1_t, x_lnT, lnp, lnps)

            # ---- P2a: Q,K GEMM -> qkT_d
            with tc.tile_pool(name="p2a", bufs=3) as p2a, \
                 tc.tile_pool(name="p2aw", bufs=8) as p2aw, \
                 tc.tile_pool(name="p2aps", bufs=4, space="PSUM") as p2aps:
                for oc in range(12):
                    wts = []
                    for cc in range(CC):
                        wt_ = p2aw.tile([128, 128], F32R, tag="wt")
                        nc.sync.dma_start(
                            out=wt_[:],
                            in_=wqkT_d[cc * 128:(cc + 1) * 128, oc * 128:(oc + 1) * 128])
                        wts.append(wt_)
                    for b in range(NBLK):
                        s, w = blk_sl(b)
                        ps_ = p2aps.tile([128, 512], F32, tag="aps")
                        for cc in range(CC):
                            nc.tensor.matmul(ps_[:, :w], wts[cc][:],
                                             x_lnT[:, cc, s:s + w],
                                             start=(cc == 0), stop=(cc == CC - 1))
                        ot = p2a.tile([128, 512], F32R, tag="aot")
                        nc.scalar.activation(out=ot[:, :w], in_=ps_[:, :w],
                                             func=AF.Identity,
                                             bias=qkb_col[:, oc, :], scale=1.0)
                        nc.sync.dma_start(
                            out=_ap(qkT_d, oc * 128 * QKT_STRIDE + s,
                                    [[QKT_STRIDE, 128], [1, w]]),
                            in_=ot[:, :w])

            # ---- P2b: V GEMM -> v_tm_d
            with tc.tile_pool(name="p2b", bufs=3) as p2b, \
                 tc.tile_pool(name="p2bps", bufs=3, space="PSUM") as p2bps:
                for tci in range(33):
                    ntok = 128 if tci < 32 else 64
                    ps_ = p2bps.tile([128, C], F32, tag="bps")
                    for nch, (ns, nw) in enumerate(((0, 512), (512, 256))):
                        for cc in range(CC):
                            nc.tensor.matmul(
                                ps_[:ntok, ns:ns + nw],
                                x_lnT[:, cc, tci * 128: tci * 128 + ntok],
                                wvT[:, cc, ns:ns + nw],
                                start=(cc == 0), stop=(cc == CC - 1))
                    ot = p2b.tile([128, C], F32R, tag="bot")
                    nc.vector.scalar_tensor_tensor(
                        out=ot[:ntok], in0=ps_[:ntok], scalar=1.0,
                        in1=bv_t[:ntok], op0=OP.mult, op1=OP.add)
                    nc.sync.dma_start(out=v_tm_d[tci * 128: tci * 128 + ntok, :],
                                      in_=ot[:ntok])

            # ---- P2c: rel GEMMs -> relh_d / relw_d
            with tc.tile_pool(name="p2c", bufs=3) as p2c, \
                 tc.tile_pool(name="p2cps", bufs=3, space="PSUM") as p2cps:
                pbbc = None
                for r in range(GR):          # rel_h, per grid row, class h=r%14
                    h = r % WS
                    ps_ = p2cps.tile([128, REL], F32, tag="cps")
                    for cc in range(CC):
                        rhs = _ap(GrT[:], cc * (NH * 54) + (13 - h),
                                  [[CC * NH * 54, 128], [54, NH], [1, WS]])
                        nc.tensor.matmul(ps_[:64, :],
                                         x_lnT[:, cc, r * 64:(r + 1) * 64],
                                         rhs, start=(cc == 0), stop=(cc == CC - 1))
                    ot = p2c.tile([128, REL], F32, tag="cot")
                    pbbc = p2c.tile([128, WS, NH], F32, tag="pbbc")
                    nc.sync.dma_start(
                        out=pbbc[:64], in_=_ap(pb_d, (13 + h) * NH,
                                               [[0, 64], [-NH, WS], [1, NH]]))
                    nc.vector.scalar_tensor_tensor(
                        out=ot[:64].rearrange("p (a b) -> p a b", a=NH),
                        in0=ps_[:64].rearrange("p (a b) -> p a b", a=NH),
                        scalar=1.0,
                        in1=_ap(pbbc[:], 0, [[WS * NH, 64], [1, NH], [NH, WS]]),
                        op0=OP.mult, op1=OP.add)
                    nc.sync.dma_start(out=relh_d[r * 64:(r + 1) * 64, :], in_=ot[:64])
                for c in range(GC):          # rel_w, per grid col, class w=c%14
                    w_ = c % WS
                    ps_ = p2cps.tile([128, REL], F32, tag="cps")
                    for cc in range(CC):
                        rhs = _ap(GrT[:], cc * (NH * 54) + 27 + (13 - w_),
                                  [[CC * NH * 54, 128], [54, NH], [1, WS]])
                        lhs = _ap(x_lnT[:], cc * T + c, [[CC * T, 128], [64, GR]])
                        nc.tensor.matmul(ps_[:GR, :], lhs, rhs,
                                         start=(cc == 0), stop=(cc == CC - 1))
                    ot = p2c.tile([128, REL], F32, tag="cot")
                    pbbc = p2c.tile([128, WS, NH], F32, tag="pbbc")
                    nc.sync.dma_start(
                        out=pbbc[:GR], in_=_ap(pb_d, (13 + w_) * NH + 27 * NH,
                                               [[0, GR], [-NH, WS], [1, NH]]))
                    nc.vector.scalar_tensor_tensor(
                        out=ot[:GR].rearrange("p (a b) -> p a b", a=NH),
                        in0=ps_[:GR].rearrange("p (a b) -> p a b", a=NH),
                        scalar=1.0,
                        in1=_ap(pbbc[:], 0, [[WS * NH, GR], [1, NH], [NH, WS]]),
                        op0=OP.mult, op1=OP.add)
                    nc.sync.dma_start(
                        out=_ap(relw_d, c * REL, [[GC * REL, GR], [1, REL]]),
                        in_=ot[:GR])

        # =========== P3: windowed attention ===========
        with tc.tile_pool(name="w_qk", bufs=1) as w_qk, \
             tc.tile_pool(name="w_v", bufs=1) as w_v, \
             tc.tile_pool(name="w_rel", bufs=2) as w_rel, \
             tc.tile_pool(name="w_att", bufs=1) as w_att, \
             tc.tile_pool(name="w_sm", bufs=1) as w_sm, \
             tc.tile_pool(name="w_x2", bufs=2) as w_x2, \
             tc.tile_pool(name="ps_s", bufs=2, space="PSUM") as ps_s, \
             tc.tile_pool(name="ps_t", bufs=2, space="PSUM") as ps_t, \
             tc.tile_pool(name="ps_av", bufs=2, space="PSUM") as ps_av, \
             tc.tile_pool(name="ps_p", bufs=1, space="PSUM") as ps_p:

            for wi in range(NWIN):
                for wj in range(NWIN):
                    nr = WS if wi < 4 else GR - 4 * WS   # 9
                    ncol = WS if wj < 4 else GC - 4 * WS  # 8
                    edge = nr < WS or ncol < WS
                    tok0 = (wi * WS) * GC + wj * WS

                    # ---- gather slabs
                    qs = w_qk.tile([128, CC, N], F32R, tag="qs")
                    ks = w_qk.tile([128, CC, N], F32R, tag="ks")
                    if edge:
                        nc.vector.tensor_copy(
                            ks[:], _ap(kb_col[:], 0,
                                       [[CC, 128], [1, CC], [0, N]]))
                    for slab, od0 in ((qs, 0), (ks, C)):
                        for cci in range(CC):
                            nc.sync.dma_start(
                                out=_ap(slab[:], cci * N,
                                        [[CC * N, 128], [WS, nr], [1, ncol]]),
                                in_=_ap(qkT_d, (od0 + cci * 128) * QKT_STRIDE + tok0,
                                        [[QKT_STRIDE, 128], [GC, nr], [1, ncol]]))

                    vs = w_v.tile([128, 2, C], F32R, tag="vs")
                    sc = w_v.tile([128, 2, C], F32, tag="sc")
                    rh = w_rel.tile([128, 2, NH, WS], F32, tag="rh")
                    rw = w_rel.tile([128, 2, NH, WS], F32, tag="rw")
                    if edge:
                        nc.sync.dma_start(
                            out=vs[:98, :, :].rearrange("p a b -> p (a b)"),
                            in_=_ap(bvr_d, 0, [[0, 98], [0, 2], [1, C]]))
                        nc.vector.memset(sc[:], 0.0)
                    for qc in range(2):
                        r0 = qc * 7
                        vr = max(0, min(7, nr - r0))
                        if vr == 0:
                            continue
                        gtok = tok0 + r0 * GC

                        def gath(slab, src_d, width, r0=r0, vr=vr, gtok=gtok, qc=qc):
                            if not edge:
                                nc.sync.dma_start(
                                    out=slab[:98, qc, :],
                                    in_=_ap(src_d, gtok * width,
                                            [[GC * width, vr], [width, WS],
                                             [1, width]]))
                            else:
                                for r in range(vr):
                                    nc.sync.dma_start(
                                        out=slab[r * WS: r * WS + ncol, qc, :],
                                        in_=_ap(src_d, (gtok + r * GC) * width,
                                                [[width, ncol], [1, width]]))

                        gath(vs, v_tm_d, C)
                        gath(rh, relh_d, REL)
                        gath(rw, relw_d, REL)
                        # shortcut: grid row 0 is prompt
                        if wi == 0 and qc == 0:
                            nc.sync.dma_start(
                                out=sc[0:ncol, 0, :],
                                in_=prompt_b[wj * WS: wj * WS + ncol, :])
                            for r in range(1, vr):
                                nc.sync.dma_start(
                                    out=sc[r * WS: r * WS + ncol, 0, :],
                                    in_=x_b[(r - 1) * GC + wj * WS:
                                            (r - 1) * GC + wj * WS + ncol, :])
                        else:
                            xr0 = (wi * WS + r0 - 1) * GC + wj * WS
                            if not edge:
                                nc.sync.dma_start(
                                    out=sc[:98, qc, :],
                                    in_=_ap(x_b, xr0 * C,
                                            [[GC * C, vr], [C, WS], [1, C]]))
                            else:
                                for r in range(vr):
                                    nc.sync.dma_start(
                                        out=sc[r * WS: r * WS + ncol, qc, :],
                                        in_=_ap(x_b, (xr0 + r * GC) * C,
                                                [[C, ncol], [1, C]]))

                    # ---- scores + rel + softmax
                    att = w_att.tile([128, 2, NH, N], F32, tag="att")
                    probs = w_sm.tile([128, 2, NH, N], F32R, tag="probs")
                    for qc in range(2):
                        for h in range(NH):
                            pp = 64 * (h % 2)
                            hh = h // 2
                            sps = ps_s.tile([128, N], F32, tag="sps")
                            nc.tensor.matmul(
                                sps[:98, :],
                                _ap(qs[:], pp * (CC * N) + hh * N + qc * 98,
                                    [[CC * N, 64], [1, 98]]),
                                _ap(ks[:], pp * (CC * N) + hh * N,
                                    [[CC * N, 64], [1, N]]),
                                start=True, stop=True)
                            nc.vector.scalar_tensor_tensor(
                                out=att[:98, qc, h, :].rearrange("p (a b) -> p a b", a=WS),
                                in0=sps[:98, :].rearrange("p (a b) -> p a b", a=WS),
                                scalar=float(HD) ** -0.5,
                                in1=_ap(rh[:], qc * (NH * WS) + h * WS,
                                        [[2 * NH * WS, 98], [1, WS], [0, WS]]),
                                op0=OP.mult, op1=OP.add)
                        nc.vector.tensor_add(
                            att[:98, qc].rearrange("p a (b c) -> p a b c", b=WS),
                            att[:98, qc].rearrange("p a (b c) -> p a b c", b=WS),
                            _ap(rw[:], qc * (NH * WS),
                                [[2 * NH * WS, 98], [WS, NH], [0, WS], [1, WS]]))
                        nc.scalar.activation(out=probs[:98, qc], in_=att[:98, qc],
                                             func=AF.Exp)
                    sums = w_sm.tile([128, 2, NH], F32, tag="sums")
                    for qc in range(2):
                        nc.vector.reduce_sum(sums[:98, qc], probs[:98, qc], axis=AX.X)
                    nc.vector.reciprocal(
                        sums[:98].rearrange("p a b -> p (a b)"),
                        sums[:98].rearrange("p a b -> p (a b)"))
                    for qc in range(2):
                        nc.vector.scalar_tensor_tensor(
                            out=probs[:98, qc], in0=probs[:98, qc], scalar=1.0,
                            in1=_ap(sums[:], qc * NH, [[2 * NH, 98], [1, NH], [0, N]]),
                            op0=OP.mult, op1=OP.mult)

                    # ---- transpose probs, AV, proj
                    pT = w_sm.tile([128, 2, NH, N], F32R, tag="pT")
                    for qc in range(2):
                        for h in range(NH):
                            for kc in range(2):
                                tps = ps_t.tile([128, 98], F32R, tag="tps")
                                nc.tensor.transpose(
                                    tps[:98, :98],
                                    _ap(probs[:], qc * (NH * N) + h * N + kc * 98,
                                        [[2 * NH * N, 98], [1, 98]]),
                                    ident_r[:98, :98])
                                nc.scalar.copy(
                                    _ap(pT[:], kc * (NH * N) + h * N + qc * 98,
                                        [[2 * NH * N, 98], [1, 98]]),
                                    tps[:98, :98])

                    aoT = w_att.tile([128, CC, N], F32R, tag="aoT")
                    for h in range(NH):
                        avp = ps_av.tile([64, N], F32, tag="avp")
                        for kc in range(2):
                            nc.tensor.matmul(
                                avp[:, :],
                                _ap(vs[:], kc * C + h * 64, [[2 * C, 98], [1, 64]]),
                                _ap(pT[:], kc * (NH * N) + h * N,
                                    [[2 * NH * N, 98], [1, N]]),
                                start=(kc == 0), stop=(kc == 1))
                        nc.scalar.copy(
                            _ap(aoT[:], 64 * (h % 2) * (CC * N) + (h // 2) * N,
                                [[CC * N, 64], [1, N]]),
                            avp[:, :])

                    for qc in range(2):
                        pps = ps_p.tile([128, C], F32, tag="pps")
                        for nch, (ns, nw) in enumerate(((0, 512), (512, 256))):
                            for cc in range(CC):
                                nc.tensor.matmul(
                                    pps[:98, ns:ns + nw],
                                    _ap(aoT[:], cc * N + qc * 98,
                                        [[CC * N, 128], [1, 98]]),
                                    projWT[:, cc, ns:ns + nw],
                                    start=(cc == 0), stop=(cc == CC - 1))
                        x2 = w_x2.tile([128, C], F32, tag="x2")
                        nc.vector.scalar_tensor_tensor(
                            out=x2[:98], in0=pps[:98], scalar=1.0,
                            in1=sc[:98, qc, :], op0=OP.mult, op1=OP.add)
                        nc.vector.tensor_add(x2[:98], x2[:98], projb_t[:98])
                        nc.sync.dma_start(
                            out=_ap(x2p_d, ((wi * WS + qc * 7) * 70 + wj * WS) * C,
                                    [[70 * C, 7], [C, WS], [1, C]]),
                            in_=x2[:98])

        # =========== P4+P5: LN2 + MLP1 (x_ln2T scope) ===========
        with tc.tile_pool(name="xln2T_p", bufs=1) as xln2T_p:
            x_ln2T = xln2T_p.tile([128, CC, T], F32R)
            with tc.tile_pool(name="ln2", bufs=3) as lnp2, \
                 tc.tile_pool(name="ln2ps", bufs=4, space="PSUM") as lnps2:
                tiles = []
                for ti in range(33):
                    ntok = 128 if ti < 32 else 64
                    g0 = ti * 2  # grid row
                    def mk(g0=g0, ntok=ntok):
                        def f(xt_tile):
                            nc.sync.dma_start(
                                out=xt_tile[:ntok, :],
                                in_=_ap(x2p_d, g0 * 70 * C,
                                        [[70 * C, (ntok + 63) // 64], [C, GC], [1, C]]))
                        return f
                    tiles.append((ntok, mk()))
                ln_pass(tiles, g2_t, b2_t, x_ln2T, lnp2, lnps2)

            with tc.tile_pool(name="p5", bufs=3) as p5, \
                 tc.tile_pool(name="p5w", bufs=8) as p5w, \
                 tc.tile_pool(name="p5ps", bufs=4, space="PSUM") as p5ps:
                for oc in range(24):
                    wts = []
                    for cc in range(CC):
                        wt_ = p5w.tile([128, 128], F32R, tag="wt")
                        nc.sync.dma_start(
                            out=wt_[:],
                            in_=w1T_d[cc * 128:(cc + 1) * 128, oc * 128:(oc + 1) * 128])
                        wts.append(wt_)
                    for b in range(NBLK):
                        s, w = blk_sl(b)
                        ps_ = p5ps.tile([128, 512], F32, tag="mps")
                        for cc in range(CC):
                            nc.tensor.matmul(ps_[:, :w], wts[cc][:],
                                             x_ln2T[:, cc, s:s + w],
                                             start=(cc == 0), stop=(cc == CC - 1))
                        ot = p5.tile([128, 512], F32R, tag="mot")
                        nc.scalar.activation(out=ot[:, :w], in_=ps_[:, :w],
                                             func=AF.Gelu,
                                             bias=b1_col[:, oc, :], scale=1.0)
                        nc.sync.dma_start(
                            out=_ap(h1T_d, oc * 128 * T + s, [[T, 128], [1, w]]),
                            in_=ot[:, :w])

        # =========== P6: MLP2 + residual -> out ===========
        with tc.tile_pool(name="p6w2", bufs=1) as p6w2, \
             tc.tile_pool(name="p6", bufs=3) as p6, \
             tc.tile_pool(name="p6h", bufs=26) as p6h, \
             tc.tile_pool(name="p6ps", bufs=3, space="PSUM") as p6ps:
            w2T = p6w2.tile([128, 24, C], F32R)
            for kc in range(24):
                nc.sync.dma_start(out=w2T[:, kc, :],
                                  in_=w2T_d[kc * 128:(kc + 1) * 128, :])
            for tci in range(33):
                ntok = 128 if tci < 32 else 64
                g0 = tci * 2
                x2b = p6.tile([128, C], F32, tag="x2b")
                nc.sync.dma_start(
                    out=x2b[:ntok, :],
                    in_=_ap(x2p_d, g0 * 70 * C,
                            [[70 * C, (ntok + 63) // 64], [C, GC], [1, C]]))
                nc.vector.tensor_add(x2b[:ntok], x2b[:ntok], mlpb2_t[:ntok])
                hts = []
                for kc in range(24):
                    ht = p6h.tile([128, 128], F32R, tag="ht")
                    nc.sync.dma_start(
                        out=ht[:, :ntok],
                        in_=_ap(h1T_d, kc * 128 * T + tci * 128, [[T, 128], [1, ntok]]))
                    hts.append(ht)
                ps_ = p6ps.tile([128, C], F32, tag="ops")
                for ns, nw in ((0, 512), (512, 256)):
                    for kc in range(24):
                        nc.tensor.matmul(ps_[:ntok, ns:ns + nw],
                                         hts[kc][:, :ntok], w2T[:, kc, ns:ns + nw],
                                         start=(kc == 0), stop=(kc == 23))
                ot = p6.tile([128, C], F32, tag="oot")
                nc.vector.scalar_tensor_tensor(
                    out=ot[:ntok], in0=ps_[:ntok], scalar=1.0,
                    in1=x2b[:ntok], op0=OP.mult, op1=OP.add)
                nc.sync.dma_start(out=out[tci * 128: tci * 128 + ntok, :],
                                  in_=ot[:ntok])

    nc.compile()
    return nc


def _get_nc():
    if "nc" not in _CACHE:
        _CACHE["nc"] = build()
    return _CACHE["nc"]


def kernel(**inputs):
    nc = _get_nc()
    f = np.float32
    shared = {
        "ln1_g": inputs["ln1_g"].reshape(1, C).astype(f),
        "ln1_b": inputs["ln1_b"].reshape(1, C).astype(f),
        "qkv_w": np.ascontiguousarray(inputs["qkv_w"], dtype=f),
        "qkv_b": inputs["qkv_b"].reshape(3 * C, 1).astype(f),
        "rel_h_in": np.ascontiguousarray(inputs["rel_pos_h"], dtype=f),
        "rel_w_in": np.ascontiguousarray(inputs["rel_pos_w"], dtype=f),
        "proj_w": np.ascontiguousarray(inputs["proj_w"], dtype=f),
        "proj_b": inputs["proj_b"].reshape(1, C).astype(f),
        "ln2_g": inputs["ln2_g"].reshape(1, C).astype(f),
        "ln2_b": inputs["ln2_b"].reshape(1, C).astype(f),
        "mlp_w1": np.ascontiguousarray(inputs["mlp_w1"], dtype=f),
        "mlp_b1": inputs["mlp_b1"].reshape(MLP, 1).astype(f),
        "mlp_w2": np.ascontiguousarray(inputs["mlp_w2"], dtype=f),
        "mlp_b2": inputs["mlp_b2"].reshape(1, C).astype(f),
    }
    x = np.asarray(inputs["x"], dtype=f)
    pe = np.asarray(inputs["prompt_emb"], dtype=f)
    in_maps = []
    for b in range(8):
        m = dict(shared)
        m["x_b"] = np.ascontiguousarray(x[b].reshape(4096, C))
        m["prompt_b"] = np.ascontiguousarray(pe[b, 0])
        in_maps.append(m)
    res = run_bass_kernel_spmd(nc, in_maps, list(range(8))).results
    return np.stack([r["out"].reshape(GR, GC, C) for r in res])


if __name__ == "__main__":
    nc = build()
    print("build ok")


# revision 30
# speedup vs baseline: 1.0393x; 1.0393x over previous
"""SAM-style windowed-attention transformer block on 8 TRN2 cores.

Data-parallel: core b processes batch element b end-to-end (no collectives).
Shapes hardcoded: x [8,64,64,768], prompt row prepended -> grid 65x64 tokens,
windows 14x14 (5x5=25 windows, padded grid 70x70), NH=12, HD=64, MLP=3072.
"""
import numpy as np
from contextlib import ExitStack

import concourse.bass as bass
import concourse.tile as tile
import concourse.mybir as mybir
from concourse import bacc
from concourse.bass_utils import run_bass_kernel_spmd
from concourse.masks import make_identity

F32 = mybir.dt.float32
F32R = mybir.dt.float32r
BF16 = mybir.dt.bfloat16
AF = mybir.ActivationFunctionType
OP = mybir.AluOpType
AX = mybir.AxisListType

WS = 14
NH = 12
HD = 64
C = 768
MLP = 3072
GR, GC = 65, 64          # token grid (prompt row + 64 image rows) x 64 cols
T = GR * GC              # 4160 tokens
NWIN = 5                 # windows per axis (70/14)
N = WS * WS              # 196 tokens per window
CC = C // 128            # 6 contraction chunks
QK_OD = 2 * C            # q then k, feature-major
REL = NH * WS            # 168 rel output dims (head, k-index)
QKT_STRIDE = 4224        # 4160 padded to avoid OOB on wrapped edge reads

_CACHE = {}


def _ap(base, off, dims):
    return bass.AP(tensor=base.tensor, offset=base.offset + off, ap=dims)


def build():
    nc = bacc.Bacc("TRN2", target_bir_lowering=False, debug=False)

    # ---------------- I/O ----------------
    x_b = nc.dram_tensor("x_b", [4096, C], F32, kind="ExternalInput").ap()
    prompt_b = nc.dram_tensor("prompt_b", [64, C], F32, kind="ExternalInput").ap()
    ln1_g = nc.dram_tensor("ln1_g", [1, C], F32, kind="ExternalInput").ap()
    ln1_b = nc.dram_tensor("ln1_b", [1, C], F32, kind="ExternalInput").ap()
    qkv_w = nc.dram_tensor("qkv_w", [3 * C, C], F32, kind="ExternalInput").ap()
    qkv_b = nc.dram_tensor("qkv_b", [3 * C, 1], F32, kind="ExternalInput").ap()
    rel_h_in = nc.dram_tensor("rel_h_in", [27, HD], F32, kind="ExternalInput").ap()
    rel_w_in = nc.dram_tensor("rel_w_in", [27, HD], F32, kind="ExternalInput").ap()
    proj_w = nc.dram_tensor("proj_w", [C, C], F32, kind="ExternalInput").ap()
    proj_b = nc.dram_tensor("proj_b", [1, C], F32, kind="ExternalInput").ap()
    ln2_g = nc.dram_tensor("ln2_g", [1, C], F32, kind="ExternalInput").ap()
    ln2_b = nc.dram_tensor("ln2_b", [1, C], F32, kind="ExternalInput").ap()
    mlp_w1 = nc.dram_tensor("mlp_w1", [MLP, C], F32, kind="ExternalInput").ap()
    mlp_b1 = nc.dram_tensor("mlp_b1", [MLP, 1], F32, kind="ExternalInput").ap()
    mlp_w2 = nc.dram_tensor("mlp_w2", [C, MLP], F32, kind="ExternalInput").ap()
    mlp_b2 = nc.dram_tensor("mlp_b2", [1, C], F32, kind="ExternalInput").ap()
    out = nc.dram_tensor("out", [T, C], F32, kind="ExternalOutput").ap()

    # ---------------- internal DRAM ----------------
    wqkT_d = nc.dram_tensor("wqkT_d", [C, QK_OD], F32R).ap()
    w1T_d = nc.dram_tensor("w1T_d", [C, MLP], F32R).ap()
    w2T_d = nc.dram_tensor("w2T_d", [MLP, C], F32R).ap()
    qkT_d = nc.dram_tensor("qkT_d", [QK_OD, QKT_STRIDE], BF16).ap()
    v_tm_d = nc.dram_tensor("v_tm_d", [T, C], BF16).ap()
    relh_d = nc.dram_tensor("relh_d", [T, REL], F32).ap()
    relw_d = nc.dram_tensor("relw_d", [T, REL], F32).ap()
    pb_d = nc.dram_tensor("pb_d", [54, NH], F32).ap()  # rel bias table (t, head)
    x2p_d = nc.dram_tensor("x2p_d", [70, 70, C], F32).ap()
    h1T_d = nc.dram_tensor("h1T_d", [MLP, T], F32R).ap()
    bvr_d = nc.dram_tensor("bvr_d", [1, C], BF16).ap()

    NBLK = 9  # token blocks for 512-wide moving ops (8x512 + 64)

    def blk_sl(b):
        s = b * 512
        return s, min(512, T - s)

    with tile.TileContext(nc) as tc, ExitStack() as ctx0:
        persist = ctx0.enter_context(tc.tile_pool(name="persist", bufs=1))

        # identity matrices for PE transpose
        ident = persist.tile([128, 128], F32)
        make_identity(nc, ident[:])
        ident_b = persist.tile([128, 128], BF16)
        nc.vector.tensor_copy(ident_b[:], ident[:])

        # broadcast/bias tiles
        def bcast_row(src_row, w, tag, dt=F32, pool=None):
            t_ = (pool or persist).tile([128, w], dt, tag=tag)
            nc.sync.dma_start(out=t_[:], in_=_ap(src_row, 0, [[0, 128], [1, w]]))
            return t_

        g1_t = bcast_row(ln1_g, C, "g1_t")
        b1_t = bcast_row(ln1_b, C, "b1_t")
        g2_t = bcast_row(ln2_g, C, "g2_t")
        b2_t = bcast_row(ln2_b, C, "b2_t")
        projb_t = bcast_row(proj_b, C, "projb_t")
        mlpb2_t = bcast_row(mlp_b2, C, "mlpb2_t")
        bv_t = bcast_row(_ap(qkv_b, 2 * C, [[1, 1], [1, C]]), C, "bv_t")
        bv_r = persist.tile([128, C], BF16)
        nc.vector.tensor_copy(bv_r[:], bv_t[:])
        nc.sync.dma_start(out=bvr_d, in_=bv_r[0:1, :])
        qkb_col = persist.tile([128, 12, 1], F32)   # q,k bias cols per oc
        nc.sync.dma_start(out=qkb_col[:], in_=_ap(qkv_b, 0, [[1, 128], [128, 12], [0, 1]]))
        kb_col = persist.tile([128, 6, 1], F32)     # k bias cols for slab prefill
        nc.sync.dma_start(out=kb_col[:], in_=_ap(qkv_b, C, [[1, 128], [128, 6], [0, 1]]))
        b1_col = persist.tile([128, 24, 1], F32)
        nc.sync.dma_start(out=b1_col[:], in_=_ap(mlp_b1, 0, [[1, 128], [128, 24], [0, 1]]))
        eps_t = persist.tile([128, 1], F32)
        nc.vector.memset(eps_t[:], 1e-5)

        # rel_cat^T [64, 54] (transpose gather, one-time small)
        relT = persist.tile([64, 54], F32)
        nc.sync.dma_start(out=relT[:, 0:27], in_=_ap(rel_h_in, 0, [[1, 64], [64, 27]]))
        nc.sync.dma_start(out=relT[:, 27:54], in_=_ap(rel_w_in, 0, [[1, 64], [64, 27]]))
        # bq matrix [64, 12]
        bq_m = persist.tile([64, 12], F32)
        nc.sync.dma_start(out=bq_m[:], in_=_ap(qkv_b, 0, [[1, 64], [64, 12], [0, 1]]))

        projWT = persist.tile([128, CC, C], F32R)   # proj_w^T resident
        GrT = persist.tile([128, CC, NH, 54], F32R)  # [c, cc, head, revh27|revw27]

        # =========== P0: weight transposes + rel weight build ===========
        with tc.tile_pool(name="p0", bufs=3) as p0, \
             tc.tile_pool(name="p0ps", bufs=4, space="PSUM") as p0ps:

            def wtile_T(src, oc, n_oc, dst_dram, dst_sb=None):
                # natural W rows [128 od, C] -> 6 transposes -> [128 c, 128 od]
                wn = p0.tile([128, C], F32, tag="wn")
                nc.sync.dma_start(out=wn[:], in_=src[oc * 128:(oc + 1) * 128, :])
                for cc in range(CC):
                    pt = p0ps.tile([128, 128], F32, tag="pt")
                    nc.tensor.transpose(pt[:], wn[:, cc * 128:(cc + 1) * 128], ident[:])
                    if dst_sb is not None:
                        nc.scalar.copy(dst_sb[:, cc, oc * 128:(oc + 1) * 128], pt[:])
                    else:
                        ws = p0.tile([128, 128], F32R, tag="ws")
                        nc.scalar.copy(ws[:], pt[:])
                        nc.sync.dma_start(
                            out=dst_dram[cc * 128:(cc + 1) * 128,
                                         oc * 128:(oc + 1) * 128], in_=ws[:])

            for oc in range(12):            # Q,K -> DRAM
                wtile_T(qkv_w, oc, 12, wqkT_d)
            wvT = persist.tile([128, CC, C], F32R)
            for oc in range(6):             # V -> resident (od offset 1536)
                wn = p0.tile([128, C], F32, tag="wn")
                nc.sync.dma_start(out=wn[:], in_=qkv_w[2 * C + oc * 128: 2 * C + (oc + 1) * 128, :])
                for cc in range(CC):
                    pt = p0ps.tile([128, 128], F32, tag="pt")
                    nc.tensor.transpose(pt[:], wn[:, cc * 128:(cc + 1) * 128], ident[:])
                    nc.scalar.copy(wvT[:, cc, oc * 128:(oc + 1) * 128], pt[:])
            for oc in range(6):             # proj -> resident
                wn = p0.tile([128, C], F32, tag="wn")
                nc.sync.dma_start(out=wn[:], in_=proj_w[oc * 128:(oc + 1) * 128, :])
                for cc in range(CC):
                    pt = p0ps.tile([128, 128], F32, tag="pt")
                    nc.tensor.transpose(pt[:], wn[:, cc * 128:(cc + 1) * 128], ident[:])
                    nc.scalar.copy(projWT[:, cc, oc * 128:(oc + 1) * 128], pt[:])
            for oc in range(24):            # mlp_w1 -> DRAM
                wtile_T(mlp_w1, oc, 24, w1T_d)
            for oc2 in range(6):            # mlp_w2 [768, 3072] -> w2T [3072, 768]
                wn = p0.tile([128, MLP], F32, tag="wn2")
                nc.sync.dma_start(out=wn[:], in_=mlp_w2[oc2 * 128:(oc2 + 1) * 128, :])
                for cc2 in range(24):
                    pt = p0ps.tile([128, 128], F32, tag="pt")
                    nc.tensor.transpose(pt[:], wn[:, cc2 * 128:(cc2 + 1) * 128], ident[:])
                    ws = p0.tile([128, 128], F32R, tag="ws")
                    nc.scalar.copy(ws[:], pt[:])
                    nc.sync.dma_start(
                        out=w2T_d[cc2 * 128:(cc2 + 1) * 128,
                                  oc2 * 128:(oc2 + 1) * 128], in_=ws[:])

            # G matrices: per head psum[c_chunk, 54] = Wq_head-slice^T @ relT
            for h in range(NH):
                wqn = p0.tile([64, C], F32, tag="wqn")
                nc.sync.dma_start(out=wqn[:], in_=qkv_w[h * 64:(h + 1) * 64, :])
                for cc in range(CC):
                    gp = p0ps.tile([128, 54], F32, tag="gp")
                    nc.tensor.matmul(gp[:], wqn[:, cc * 128:(cc + 1) * 128],
                                     relT[:], start=True, stop=True)
                    # store reversed: dst t' = 26-t per half
                    nc.vector.tensor_copy(
                        GrT[:, cc, h, 0:27],
                        _ap(gp[:], 26, [[54, 128], [-1, 27]]))
                    nc.vector.tensor_copy(
                        GrT[:, cc, h, 27:54],
                        _ap(gp[:], 53, [[54, 128], [-1, 27]]))
            # rel bias table: psum [54 t, 12 h] = relT^T? -> lhsT=relT[64,54], rhs=bq_m
            pbp = p0ps.tile([54, 12], F32, tag="gp")
            nc.tensor.matmul(pbp[:], relT[:], bq_m[:], start=True, stop=True)
            pbs = p0.tile([54, 12], F32, tag="pbs")
            nc.scalar.copy(pbs[:], pbp[:])
            nc.sync.dma_start(out=pb_d, in_=pbs[:])

        # =========== P1..P2: LN1, big GEMMs (x_lnT scope) ===========
        with tc.tile_pool(name="xlnT_p", bufs=1) as xlnT_p:
            x_lnT = xlnT_p.tile([128, CC, T], F32R)

            def ln_pass(src_tiles, g_t, b_t, dstT, pool, psp):
                """src_tiles: list of (n_tok, dma_fn) per 128-token tile."""
                for ti, (ntok, dma_fn) in enumerate(src_tiles):
                    xt = pool.tile([128, C], F32, tag="xt")
                    dma_fn(xt)
                    st = pool.tile([128, 3, 6], F32, tag="st")
                    for sg in range(3):
                        nc.vector.bn_stats(out=st[:ntok, sg, :],
                                           in_=xt[:ntok, sg * 256:(sg + 1) * 256])
                    mv = pool.tile([128, 2], F32, tag="mv")
                    nc.vector.bn_aggr(out=mv[:ntok], in_=st[:ntok])
                    rstd = pool.tile([128, 1], F32, tag="rstd")
                    nc.scalar.activation(out=rstd[:ntok], in_=mv[:ntok, 1:2],
                                         func=AF.Sqrt, bias=eps_t[:ntok], scale=1.0)
                    nc.vector.reciprocal(out=rstd[:ntok], in_=rstd[:ntok])
                    bmn = pool.tile([128, 1], F32, tag="bmn")
                    nc.vector.scalar_tensor_tensor(
                        out=bmn[:ntok], in0=mv[:ntok, 0:1], scalar=-1.0,
                        in1=rstd[:ntok], op0=OP.mult, op1=OP.mult)
                    xn = pool.tile([128, C], F32, tag="xn")
                    nc.scalar.activation(out=xn[:ntok], in_=xt[:ntok],
                                         func=AF.Identity, bias=bmn[:ntok],
                                         scale=rstd[:ntok])
                    nc.vector.scalar_tensor_tensor(
                        out=xn[:ntok], in0=xn[:ntok], scalar=1.0,
                        in1=g_t[:ntok], op0=OP.mult, op1=OP.mult)
                    nc.vector.tensor_add(xn[:ntok], xn[:ntok], b_t[:ntok])
                    for cc in range(CC):
                        pt = psp.tile([128, 128], F32, tag="lnpt")
                        nc.tensor.transpose(pt[:, :ntok],
                                            xn[:ntok, cc * 128:(cc + 1) * 128],
                                            ident[:ntok, :ntok])
                        nc.scalar.copy(dstT[:, cc, ti * 128: ti * 128 + ntok],
                                       pt[:, :ntok])

            with tc.tile_pool(name="ln1", bufs=3) as lnp, \
                 tc.tile_pool(name="ln1ps", bufs=4, space="PSUM") as lnps:
                tiles = []

                def mk_dma0(xt_tile):
                    nc.sync.dma_start(out=xt_tile[0:64, :], in_=prompt_b)
                    nc.sync.dma_start(out=xt_tile[64:128, :], in_=x_b[0:64, :])

                tiles.append((128, mk_dma0))
                for ti in range(1, 33):
                    ntok = 128 if ti < 32 else 64
                    def mk(ti=ti, ntok=ntok):
                        def f(xt_tile):
                            nc.sync.dma_start(
                                out=xt_tile[:ntok, :],
                                in_=x_b[ti * 128 - 64: ti * 128 - 64 + ntok, :])
                        return f
                    tiles.append((ntok, mk()))
                ln_pass(tiles, g1_t, b1_t, x_lnT, lnp, lnps)

            # ---- P2a: Q,K GEMM -> qkT_d
            with tc.tile_pool(name="p2a", bufs=3) as p2a, \
                 tc.tile_pool(name="p2aw", bufs=8) as p2aw, \
                 tc.tile_pool(name="p2aps", bufs=4, space="PSUM") as p2aps:
                for oc in range(12):
                    wts = p2aw.tile([128, CC, 128], F32R, tag="wt")
                    nc.sync.dma_start(
                        out=wts[:],
                        in_=_ap(wqkT_d, oc * 128,
                                [[QK_OD, 128], [128 * QK_OD, CC], [1, 128]]))
                    for b in range(NBLK):
                        s, w = blk_sl(b)
                        ps_ = p2aps.tile([128, 512], F32, tag="aps")
                        for cc in range(CC):
                            nc.tensor.matmul(ps_[:, :w], wts[:, cc, :],
                                             x_lnT[:, cc, s:s + w],
                                             start=(cc == 0), stop=(cc == CC - 1))
                        ot = p2a.tile([128, 512], BF16, tag="aot")
                        nc.scalar.activation(out=ot[:, :w], in_=ps_[:, :w],
                                             func=AF.Identity,
                                             bias=qkb_col[:, oc, :], scale=1.0)
                        nc.sync.dma_start(
                            out=_ap(qkT_d, oc * 128 * QKT_STRIDE + s,
                                    [[QKT_STRIDE, 128], [1, w]]),
                            in_=ot[:, :w])

            # ---- P2b: V GEMM -> v_tm_d
            with tc.tile_pool(name="p2b", bufs=3) as p2b, \
                 tc.tile_pool(name="p2bps", bufs=3, space="PSUM") as p2bps:
                for tci in range(33):
                    ntok = 128 if tci < 32 else 64
                    ps_ = p2bps.tile([128, C], F32, tag="bps")
                    for nch, (ns, nw) in enumerate(((0, 512), (512, 256))):
                        for cc in range(CC):
                            nc.tensor.matmul(
                                ps_[:ntok, ns:ns + nw],
                                x_lnT[:, cc, tci * 128: tci * 128 + ntok],
                                wvT[:, cc, ns:ns + nw],
                                start=(cc == 0), stop=(cc == CC - 1))
                    ot = p2b.tile([128, C], BF16, tag="bot")
                    nc.vector.scalar_tensor_tensor(
                        out=ot[:ntok], in0=ps_[:ntok], scalar=1.0,
                        in1=bv_t[:ntok], op0=OP.mult, op1=OP.add)
                    nc.sync.dma_start(out=v_tm_d[tci * 128: tci * 128 + ntok, :],
                                      in_=ot[:ntok])

            # ---- P2c: rel GEMMs -> relh_d / relw_d
            with tc.tile_pool(name="p2c", bufs=3) as p2c, \
                 tc.tile_pool(name="p2cps", bufs=3, space="PSUM") as p2cps:
                pbbc = None
                for r in range(GR):          # rel_h, per grid row, class h=r%14
                    h = r % WS
                    ps_ = p2cps.tile([128, REL], F32, tag="cps")
                    for cc in range(CC):
                        rhs = _ap(GrT[:], cc * (NH * 54) + (13 - h),
                                  [[CC * NH * 54, 128], [54, NH], [1, WS]])
                        nc.tensor.matmul(ps_[:64, :],
                                         x_lnT[:, cc, r * 64:(r + 1) * 64],
                                         rhs, start=(cc == 0), stop=(cc == CC - 1))
                    ot = p2c.tile([128, REL], F32, tag="cot")
                    pbbc = p2c.tile([128, WS, NH], F32, tag="pbbc")
                    nc.sync.dma_start(
                        out=pbbc[:64], in_=_ap(pb_d, (13 + h) * NH,
                                               [[0, 64], [-NH, WS], [1, NH]]))
                    nc.vector.scalar_tensor_tensor(
                        out=ot[:64].rearrange("p (a b) -> p a b", a=NH),
                        in0=ps_[:64].rearrange("p (a b) -> p a b", a=NH),
                        scalar=1.0,
                        in1=_ap(pbbc[:], 0, [[WS * NH, 64], [1, NH], [NH, WS]]),
                        op0=OP.mult, op1=OP.add)
                    nc.sync.dma_start(out=relh_d[r * 64:(r + 1) * 64, :], in_=ot[:64])
                for c in range(GC):          # rel_w, per grid col, class w=c%14
                    w_ = c % WS
                    ps_ = p2cps.tile([128, REL], F32, tag="cps")
                    for cc in range(CC):
                        rhs = _ap(GrT[:], cc * (NH * 54) + 27 + (13 - w_),
                                  [[CC * NH * 54, 128], [54, NH], [1, WS]])
                        lhs = _ap(x_lnT[:], cc * T + c, [[CC * T, 128], [64, GR]])
                        nc.tensor.matmul(ps_[:GR, :], lhs, rhs,
                                         start=(cc == 0), stop=(cc == CC - 1))
                    ot = p2c.tile([128, REL], F32, tag="cot")
                    pbbc = p2c.tile([128, WS, NH], F32, tag="pbbc")
                    nc.sync.dma_start(
                        out=pbbc[:GR], in_=_ap(pb_d, (13 + w_) * NH + 27 * NH,
                                               [[0, GR], [-NH, WS], [1, NH]]))
                    nc.vector.scalar_tensor_tensor(
                        out=ot[:GR].rearrange("p (a b) -> p a b", a=NH),
                        in0=ps_[:GR].rearrange("p (a b) -> p a b", a=NH),
                        scalar=1.0,
                        in1=_ap(pbbc[:], 0, [[WS * NH, GR], [1, NH], [NH, WS]]),
                        op0=OP.mult, op1=OP.add)
                    nc.sync.dma_start(
                        out=_ap(relw_d, c * REL, [[GC * REL, GR], [1, REL]]),
                        in_=ot[:GR])

        # =========== P3: windowed attention ===========
        with tc.tile_pool(name="w_qk", bufs=2) as w_qk, \
             tc.tile_pool(name="w_v", bufs=2) as w_v, \
             tc.tile_pool(name="w_rel", bufs=2) as w_rel, \
             tc.tile_pool(name="w_att", bufs=1) as w_att, \
             tc.tile_pool(name="w_sm", bufs=2) as w_sm, \
             tc.tile_pool(name="w_x2", bufs=2) as w_x2, \
             tc.tile_pool(name="ps_s", bufs=2, space="PSUM") as ps_s, \
             tc.tile_pool(name="ps_t", bufs=2, space="PSUM") as ps_t, \
             tc.tile_pool(name="ps_av", bufs=2, space="PSUM") as ps_av, \
             tc.tile_pool(name="ps_p", bufs=1, space="PSUM") as ps_p:

            for wi in range(NWIN):
                for wj in range(NWIN):
                    nr = WS if wi < 4 else GR - 4 * WS   # 9
                    ncol = WS if wj < 4 else GC - 4 * WS  # 8
                    edge = nr < WS or ncol < WS
                    tok0 = (wi * WS) * GC + wj * WS

                    # ---- gather slabs
                    qs = w_qk.tile([128, CC, N], BF16, tag="qs")
                    ks = w_qk.tile([128, CC, N], BF16, tag="ks")
                    if edge:
                        nc.vector.tensor_copy(
                            ks[:], _ap(kb_col[:], 0,
                                       [[CC, 128], [1, CC], [0, N]]))
                    for slab, od0 in ((qs, 0), (ks, C)):
                        for cci in range(CC):
                            nc.sync.dma_start(
                                out=_ap(slab[:], cci * N,
                                        [[CC * N, 128], [WS, nr], [1, ncol]]),
                                in_=_ap(qkT_d, (od0 + cci * 128) * QKT_STRIDE + tok0,
                                        [[QKT_STRIDE, 128], [GC, nr], [1, ncol]]))

                    vs = w_v.tile([128, 2, C], BF16, tag="vs")
                    sc = w_v.tile([128, 2, C], F32, tag="sc")
                    rh = w_rel.tile([128, 2, NH, WS], F32, tag="rh")
                    rw = w_rel.tile([128, 2, NH, WS], F32, tag="rw")
                    if edge:
                        nc.sync.dma_start(
                            out=vs[:98, :, :].rearrange("p a b -> p (a b)"),
                            in_=_ap(bvr_d, 0, [[0, 98], [0, 2], [1, C]]))
                        nc.vector.memset(sc[:], 0.0)
                    for qc in range(2):
                        r0 = qc * 7
                        vr = max(0, min(7, nr - r0))
                        if vr == 0:
                            continue
                        gtok = tok0 + r0 * GC

                        def gath(slab, src_d, width, r0=r0, vr=vr, gtok=gtok, qc=qc):
                            if not edge:
                                nc.sync.dma_start(
                                    out=slab[:98, qc, :],
                                    in_=_ap(src_d, gtok * width,
                                            [[GC * width, vr], [width, WS],
                                             [1, width]]))
                            else:
                                for r in range(vr):
                                    nc.sync.dma_start(
                                        out=slab[r * WS: r * WS + ncol, qc, :],
                                        in_=_ap(src_d, (gtok + r * GC) * width,
                                                [[width, ncol], [1, width]]))

                        gath(vs, v_tm_d, C)
                        gath(rh, relh_d, REL)
                        gath(rw, relw_d, REL)
                        # shortcut: grid row 0 is prompt
                        if wi == 0 and qc == 0:
                            nc.sync.dma_start(
                                out=sc[0:ncol, 0, :],
                                in_=prompt_b[wj * WS: wj * WS + ncol, :])
                            for r in range(1, vr):
                                nc.sync.dma_start(
                                    out=sc[r * WS: r * WS + ncol, 0, :],
                                    in_=x_b[(r - 1) * GC + wj * WS:
                                            (r - 1) * GC + wj * WS + ncol, :])
                        else:
                            xr0 = (wi * WS + r0 - 1) * GC + wj * WS
                            if not edge:
                                nc.sync.dma_start(
                                    out=sc[:98, qc, :],
                                    in_=_ap(x_b, xr0 * C,
                                            [[GC * C, vr], [C, WS], [1, C]]))
                            else:
                                for r in range(vr):
                                    nc.sync.dma_start(
                                        out=sc[r * WS: r * WS + ncol, qc, :],
                                        in_=_ap(x_b, (xr0 + r * GC) * C,
                                                [[C, ncol], [1, C]]))

                    # ---- scores + rel + softmax
                    att = w_att.tile([128, 2, NH, N], F32, tag="att")
                    probs = w_sm.tile([128, 2, NH, N], BF16, tag="probs")
                    for qc in range(2):
                        for h in range(NH):
                            pp = 64 * (h % 2)
                            hh = h // 2
                            sps = ps_s.tile([128, N], F32, tag="sps")
                            nc.tensor.matmul(
                                sps[:98, :],
                                _ap(qs[:], pp * (CC * N) + hh * N + qc * 98,
                                    [[CC * N, 64], [1, 98]]),
                                _ap(ks[:], pp * (CC * N) + hh * N,
                                    [[CC * N, 64], [1, N]]),
                                start=True, stop=True)
                            nc.vector.scalar_tensor_tensor(
                                out=att[:98, qc, h, :].rearrange("p (a b) -> p a b", a=WS),
                                in0=sps[:98, :].rearrange("p (a b) -> p a b", a=WS),
                                scalar=float(HD) ** -0.5,
                                in1=_ap(rh[:], qc * (NH * WS) + h * WS,
                                        [[2 * NH * WS, 98], [1, WS], [0, WS]]),
                                op0=OP.mult, op1=OP.add)
                        nc.vector.tensor_add(
                            att[:98, qc].rearrange("p a (b c) -> p a b c", b=WS),
                            att[:98, qc].rearrange("p a (b c) -> p a b c", b=WS),
                            _ap(rw[:], qc * (NH * WS),
                                [[2 * NH * WS, 98], [WS, NH], [0, WS], [1, WS]]))
                        nc.scalar.activation(out=probs[:98, qc], in_=att[:98, qc],
                                             func=AF.Exp)
                    sums = w_sm.tile([128, 2, NH], F32, tag="sums")
                    for qc in range(2):
                        nc.vector.reduce_sum(sums[:98, qc], probs[:98, qc], axis=AX.X)
                    nc.vector.reciprocal(
                        sums[:98].rearrange("p a b -> p (a b)"),
                        sums[:98].rearrange("p a b -> p (a b)"))
                    for qc in range(2):
                        nc.vector.scalar_tensor_tensor(
                            out=probs[:98, qc], in0=probs[:98, qc], scalar=1.0,
                            in1=_ap(sums[:], qc * NH, [[2 * NH, 98], [1, NH], [0, N]]),
                            op0=OP.mult, op1=OP.mult)

                    # ---- transpose probs, AV, proj
                    pT = w_sm.tile([128, 2, NH, N], BF16, tag="pT")
                    for qc in range(2):
                        for h in range(NH):
                            for kc in range(2):
                                tps = ps_t.tile([128, 98], BF16, tag="tps")
                                nc.tensor.transpose(
                                    tps[:98, :98],
                                    _ap(probs[:], qc * (NH * N) + h * N + kc * 98,
                                        [[2 * NH * N, 98], [1, 98]]),
                                    ident_b[:98, :98])
                                nc.scalar.copy(
                                    _ap(pT[:], kc * (NH * N) + h * N + qc * 98,
                                        [[2 * NH * N, 98], [1, 98]]),
                                    tps[:98, :98])

                    aoT = w_att.tile([128, CC, N], F32R, tag="aoT")
                    for h in range(NH):
                        avp = ps_av.tile([64, N], F32, tag="avp")
                        for kc in range(2):
                            nc.tensor.matmul(
                                avp[:, :],
                                _ap(vs[:], kc * C + h * 64, [[2 * C, 98], [1, 64]]),
                                _ap(pT[:], kc * (NH * N) + h * N,
                                    [[2 * NH * N, 98], [1, N]]),
                                start=(kc == 0), stop=(kc == 1))
                        nc.scalar.copy(
                            _ap(aoT[:], 64 * (h % 2) * (CC * N) + (h // 2) * N,
                                [[CC * N, 64], [1, N]]),
                            avp[:, :])

                    for qc in range(2):
                        pps = ps_p.tile([128, C], F32, tag="pps")
                        for nch, (ns, nw) in enumerate(((0, 512), (512, 256))):
                            for cc in range(CC):
                                nc.tensor.matmul(
                                    pps[:98, ns:ns + nw],
                                    _ap(aoT[:], cc * N + qc * 98,
                                        [[CC * N, 128], [1, 98]]),
                                    projWT[:, cc, ns:ns + nw],
                                    start=(cc == 0), stop=(cc == CC - 1))
                        x2 = w_x2.tile([128, C], F32, tag="x2")
                        nc.vector.scalar_tensor_tensor(
                            out=x2[:98], in0=pps[:98], scalar=1.0,
                            in1=sc[:98, qc, :], op0=OP.mult, op1=OP.add)
                        nc.vector.tensor_add(x2[:98], x2[:98], projb_t[:98])
                        nc.sync.dma_start(
                            out=_ap(x2p_d, ((wi * WS + qc * 7) * 70 + wj * WS) * C,
                                    [[70 * C, 7], [C, WS], [1, C]]),
                            in_=x2[:98])

        # =========== P4+P5: LN2 + MLP1 (x_ln2T scope) ===========
        with tc.tile_pool(name="xln2T_p", bufs=1) as xln2T_p:
            x_ln2T = xln2T_p.tile([128, CC, T], F32R)
            with tc.tile_pool(name="ln2", bufs=3) as lnp2, \
                 tc.tile_pool(name="ln2ps", bufs=4, space="PSUM") as lnps2:
                tiles = []
                for ti in range(33):
                    ntok = 128 if ti < 32 else 64
                    g0 = ti * 2  # grid row
                    def mk(g0=g0, ntok=ntok):
                        def f(xt_tile):
                            nc.sync.dma_start(
                                out=xt_tile[:ntok, :],
                                in_=_ap(x2p_d, g0 * 70 * C,
                                        [[70 * C, (ntok + 63) // 64], [C, GC], [1, C]]))
                        return f
                    tiles.append((ntok, mk()))
                ln_pass(tiles, g2_t, b2_t, x_ln2T, lnp2, lnps2)

            with tc.tile_pool(name="p5", bufs=3) as p5, \
                 tc.tile_pool(name="p5w", bufs=8) as p5w, \
                 tc.tile_pool(name="p5ps", bufs=4, space="PSUM") as p5ps:
                for oc in range(24):
                    wts = p5w.tile([128, CC, 128], F32R, tag="wt")
                    nc.sync.dma_start(
                        out=wts[:],
                        in_=_ap(w1T_d, oc * 128,
                                [[MLP, 128], [128 * MLP, CC], [1, 128]]))
                    for b in range(NBLK):
                        s, w = blk_sl(b)
                        ps_ = p5ps.tile([128, 512], F32, tag="mps")
                        for cc in range(CC):
                            nc.tensor.matmul(ps_[:, :w], wts[:, cc, :],
                                             x_ln2T[:, cc, s:s + w],
                                             start=(cc == 0), stop=(cc == CC - 1))
                        ot = p5.tile([128, 512], F32R, tag="mot")
                        nc.scalar.activation(out=ot[:, :w], in_=ps_[:, :w],
                                             func=AF.Gelu,
                                             bias=b1_col[:, oc, :], scale=1.0)
                        nc.sync.dma_start(
                            out=_ap(h1T_d, oc * 128 * T + s, [[T, 128], [1, w]]),
                            in_=ot[:, :w])

        # =========== P6: MLP2 + residual -> out ===========
        with tc.tile_pool(name="p6w2", bufs=1) as p6w2, \
             tc.tile_pool(name="p6", bufs=3) as p6, \
             tc.tile_pool(name="p6h", bufs=3) as p6h, \
             tc.tile_pool(name="p6ps", bufs=3, space="PSUM") as p6ps:
            w2T = p6w2.tile([128, 24, C], F32R)
            for kc in range(24):
                nc.sync.dma_start(out=w2T[:, kc, :],
                                  in_=w2T_d[kc * 128:(kc + 1) * 128, :])
            for tci in range(33):
                ntok = 128 if tci < 32 else 64
                g0 = tci * 2
                x2b = p6.tile([128, C], F32, tag="x2b")
                nc.sync.dma_start(
                    out=x2b[:ntok, :],
                    in_=_ap(x2p_d, g0 * 70 * C,
                            [[70 * C, (ntok + 63) // 64], [C, GC], [1, C]]))
                nc.vector.tensor_add(x2b[:ntok], x2b[:ntok], mlpb2_t[:ntok])
                hts = p6h.tile([128, 24, 128], F32R, tag="ht")
                nc.sync.dma_start(
                    out=_ap(hts[:], 0, [[24 * 128, 128], [128, 24], [1, ntok]]),
                    in_=_ap(h1T_d, tci * 128,
                            [[T, 128], [128 * T, 24], [1, ntok]]))
                ps_ = p6ps.tile([128, C], F32, tag="ops")
                for ns, nw in ((0, 512), (512, 256)):
                    for kc in range(24):
                        nc.tensor.matmul(ps_[:ntok, ns:ns + nw],
                                         hts[:, kc, :ntok], w2T[:, kc, ns:ns + nw],
                                         start=(kc == 0), stop=(kc == 23))
                ot = p6.tile([128, C], F32, tag="oot")
                nc.vector.scalar_tensor_tensor(
                    out=ot[:ntok], in0=ps_[:ntok], scalar=1.0,
                    in1=x2b[:ntok], op0=OP.mult, op1=OP.add)
                nc.sync.dma_start(out=out[tci * 128: tci * 128 + ntok, :],
                                  in_=ot[:ntok])

    nc.compile()
    return nc


def _get_nc():
    if "nc" not in _CACHE:
        _CACHE["nc"] = build()
    return _CACHE["nc"]


def kernel(**inputs):
    nc = _get_nc()
    f = np.float32
    shared = {
        "ln1_g": inputs["ln1_g"].reshape(1, C).astype(f),
        "ln1_b": inputs["ln1_b"].reshape(1, C).astype(f),
        "qkv_w": np.ascontiguousarray(inputs["qkv_w"], dtype=f),
        "qkv_b": inputs["qkv_b"].reshape(3 * C, 1).astype(f),
        "rel_h_in": np.ascontiguousarray(inputs["rel_pos_h"], dtype=f),
        "rel_w_in": np.ascontiguousarray(inputs["rel_pos_w"], dtype=f),
        "proj_w": np.ascontiguousarray(inputs["proj_w"], dtype=f),
        "proj_b": inputs["proj_b"].reshape(1, C).astype(f),
        "ln2_g": inputs["ln2_g"].reshape(1, C).astype(f),
        "ln2_b": inputs["ln2_b"].reshape(1, C).astype(f),
        "mlp_w1": np.ascontiguousarray(inputs["mlp_w1"], dtype=f),
        "mlp_b1": inputs["mlp_b1"].reshape(MLP, 1).astype(f),
        "mlp_w2": np.ascontiguousarray(inputs["mlp_w2"], dtype=f),
        "mlp_b2": inputs["mlp_b2"].reshape(1, C).astype(f),
    }
    x = np.asarray(inputs["x"], dtype=f)
    pe = np.asarray(inputs["prompt_emb"], dtype=f)
    in_maps = []
    for b in range(8):
        m = dict(shared)
        m["x_b"] = np.ascontiguousarray(x[b].reshape(4096, C))
        m["prompt_b"] = np.ascontiguousarray(pe[b, 0])
        in_maps.append(m)
    res = run_bass_kernel_spmd(nc, in_maps, list(range(8))).results
    return np.stack([r["out"].reshape(GR, GC, C) for r in res])


if __name__ == "__main__":
    nc = build()
    print("build ok")
